# revision 1
# baseline (speedup 1.0000x reference)
"""Cross-modal attention kernel for Trainium2 (Bass/Tile), 8-core SPMD.

Reference computation (per batch b):
  q = Wq @ U + bq            U = unet_feat[b]  reshaped [320, 4096]
  k = Wk @ J + bk            J = janus_feat[b] reshaped [1024, 4096]
  v = Wv @ J + bv
  P = softmax(q^T k / 16, axis=keys)
  O = v @ P^T
  out = U + Wo @ O + bo

Sharding: 8 cores = 4 batches x 2 query-halves. Each core computes K/V for
its batch's full key set and attention for its half of the queries.

On-chip layout (per core):
  K   [C=256, N=4096]   (2 partition chunks)     f32r
  V^T [N=4096, Cv=321]  (32 partition chunks, last col = ones -> denominators)
  Q   [C=256, NQ=2048]  f32r
  S^T = K^T Q computed per (key-chunk, q-tile) in PSUM, exp'd on ACT into
  E^T [4096, 512] f32r, then O_aug = V~^T.T @ E^T accumulated in PSUM.
  Softmax needs no max-subtraction: |scores/16| <~ 1.5 by construction.

All matmuls use float32r (full PE rate at free-dim >= 256, ~1e-4 rel err).
"""
import sys

if "/opt/trn_rl_repo" not in sys.path:
    sys.path.insert(0, "/opt/trn_rl_repo")

import numpy as np

import concourse.bass as bass
import concourse.bacc as bacc
import concourse.mybir as mybir
import concourse.tile as tile

F32 = mybir.dt.float32
F32R = mybir.dt.float32r
AF = mybir.ActivationFunctionType

B = 4
C = 256        # ATTN_DIM
CU = 320
CJ = 1024
N = 4096       # H*W
NQ = N // 2    # queries per core
QT = 512       # query tile
NKT = 256      # key tile during projections
SCALE = C ** -0.5
NCORES = 8


def build_program():
    nc = bacc.Bacc("TRN2", target_bir_lowering=False, debug=False)

    u = nc.dram_tensor("u", (CU, NQ), F32, kind="ExternalInput")
    jf = nc.dram_tensor("jf", (CJ, N), F32, kind="ExternalInput")
    # weights arrive pre-transposed from the host (lhsT layout)
    wqT = nc.dram_tensor("wqT", (CU, C), F32, kind="ExternalInput")
    bq = nc.dram_tensor("bq", (C,), F32, kind="ExternalInput")
    wkT = nc.dram_tensor("wkT", (CJ, C), F32, kind="ExternalInput")
    bk = nc.dram_tensor("bk", (C,), F32, kind="ExternalInput")
    wvT = nc.dram_tensor("wvT", (CJ, CU), F32, kind="ExternalInput")
    bv = nc.dram_tensor("bv", (CU,), F32, kind="ExternalInput")
    woT = nc.dram_tensor("woT", (CU, CU), F32, kind="ExternalInput")
    bo = nc.dram_tensor("bo", (CU,), F32, kind="ExternalInput")
    out = nc.dram_tensor("out", (CU, NQ), F32, kind="ExternalOutput")

    with tile.TileContext(nc) as tc:
        with tc.tile_pool(name="perm", bufs=1) as perm:
            # ---- persistent tiles ----
            K_sb = perm.tile([128, 2, N], F32R, name="K_sb")
            Vt_sb = perm.tile([128, 32, CU + 1], F32R, name="Vt_sb")
            Q_sb = perm.tile([128, 2, NQ], F32R, name="Q_sb")
            U_sb = perm.tile([128, 3, NQ], F32, name="U_sb")
            WoT = perm.tile([128, 3, CU], F32R, name="WoT")
            bo_sb = perm.tile([128, 3], F32, name="bo_sb")
            ones_mat = perm.tile([128, 128], F32R, name="ones_mat")
            dsb = perm.tile([128, QT], F32R, name="dsb")

            # constants: build in fp32, cast to f32r so walrus sees a rounded
            # producer for every f32r matmul operand
            ones32 = perm.tile([128, 128], F32, name="ones32")
            nc.vector.memset(ones32[:, :], 1.0)
            nc.vector.tensor_copy(ones_mat[:, :], ones32[:, :])
            nc.vector.tensor_copy(Vt_sb[:, :, CU], ones32[:, 0:32])
            zeros32 = perm.tile([128, QT], F32, name="zeros32")
            nc.vector.memset(zeros32[:, :], 0.0)
            nc.vector.tensor_copy(dsb[:, :], zeros32[:, :])
            nc.vector.memset(U_sb[:, 2, :], 0.0)

            for m in range(3):
                msz = min(128, CU - m * 128)
                nc.sync.dma_start(U_sb[0:msz, m, :], u[m * 128:m * 128 + msz, :])
                nc.sync.dma_start(bo_sb[0:msz, m:m + 1],
                                  bo[m * 128:m * 128 + msz].unsqueeze(1))

            # ================= phase 1: weights + projections =================
            with tc.tile_pool(name="psb", bufs=1) as psb:
                WqT = psb.tile([128, 3, C], F32R, name="WqT")
                WkT = psb.tile([128, 8, C], F32R, name="WkT")
                WvT = psb.tile([128, 8, CU], F32R, name="WvT")
                bq_sb = psb.tile([128, 2], F32, name="bq_sb")
                bk_sb = psb.tile([128, 2], F32, name="bk_sb")
                bv_bc = psb.tile([128, CU], F32, name="bv_bc")

                for m in range(2):
                    nc.sync.dma_start(bq_sb[:, m:m + 1],
                                      bq[m * 128:(m + 1) * 128].unsqueeze(1))
                    nc.sync.dma_start(bk_sb[:, m:m + 1],
                                      bk[m * 128:(m + 1) * 128].unsqueeze(1))
                bv_ap = bv[:]
                bv_bcast = bass.AP(tensor=bv_ap.tensor, offset=bv_ap.offset,
                                   ap=[[0, 128], bv_ap.ap[0]])
                nc.sync.dma_start(bv_bc[:, :], bv_bcast)

                # --- load pre-transposed weights, cast fp32 -> f32r ---
                with tc.tile_pool(name="wstage", bufs=1) as wst:
                    def load_wT(wT_dram, Cd, O, WT, tag):
                        cch = (Cd + 127) // 128
                        w_sb = wst.tile([128, cch, O], F32, name=f"w_{tag}")
                        if Cd % 128 == 0:
                            nc.sync.dma_start(
                                w_sb[:, :, :],
                                wT_dram[:, :].rearrange("(c p) o -> p c o", p=128))
                        else:
                            for c in range(cch):
                                csz = min(128, Cd - c * 128)
                                nc.sync.dma_start(
                                    w_sb[0:csz, c, :],
                                    wT_dram[c * 128:c * 128 + csz, :])
                        for c in range(cch):
                            csz = min(128, Cd - c * 128)
                            nc.vector.tensor_copy(WT[0:csz, c, :],
                                                  w_sb[0:csz, c, :])

                    load_wT(wqT, CU, C, WqT, "wq")
                    load_wT(wkT, CJ, C, WkT, "wk")
                    load_wT(wvT, CJ, CU, WvT, "wv")
                    load_wT(woT, CU, CU, WoT, "wo")

                # --- K and V^T projections, streaming J in NKT-column tiles ---
                with tc.tile_pool(name="jp", bufs=2) as jp, \
                     tc.tile_pool(name="pps", bufs=2, space="PSUM") as pps:
                    j_r = jf[:, :].rearrange("(c p) n -> p c n", p=128)
                    for t in range(N // NKT):
                        jt = jp.tile([128, 8, NKT], F32, name="jt")
                        nc.sync.dma_start(jt[:, :, :],
                                          j_r[:, :, t * NKT:(t + 1) * NKT])
                        jr = jp.tile([128, 8, NKT], F32R, name="jr")
                        nc.scalar.copy(jr[:, :, :], jt[:, :, :])
                        # K[:, tile] = Wk^T.T @ J + bk
                        for m in range(2):
                            pk = pps.tile([128, NKT], F32, name="pk", tag="pk",
                                          padded_shape=[128, QT])
                            for cc in range(8):
                                nc.tensor.matmul(pk[:, :],
                                                 WkT[:, cc, m * 128:(m + 1) * 128],
                                                 jr[:, cc, :],
                                                 start=(cc == 0), stop=(cc == 7))
                            nc.vector.tensor_scalar_add(
                                K_sb[:, m, t * NKT:(t + 1) * NKT], pk[:, :],
                                bk_sb[:, m:m + 1])
                        # V^T[tile, :] = J.T @ Wv^T + bv
                        for s in range(NKT // 128):
                            nkc = t * (NKT // 128) + s
                            pv = pps.tile([128, CU], F32, name="pv", tag="pv")
                            for cc in range(8):
                                nc.tensor.matmul(pv[:, :],
                                                 jr[:, cc, s * 128:(s + 1) * 128],
                                                 WvT[:, cc, :],
                                                 start=(cc == 0), stop=(cc == 7))
                            nc.vector.tensor_add(Vt_sb[:, nkc, 0:CU], pv[:, :],
                                                 bv_bc[:, :])
                    # --- Q projection ---
                    for si in range(NQ // QT):
                        ur = jp.tile([128, 3, QT], F32R, name="ur")
                        nc.vector.tensor_copy(ur[:, :, :],
                                              U_sb[:, :, si * QT:(si + 1) * QT])
                        for m in range(2):
                            pq = pps.tile([128, QT], F32, name="pq", tag="pk")
                            for cc in range(3):
                                csz = min(128, CU - cc * 128)
                                nc.tensor.matmul(pq[:, :],
                                                 WqT[0:csz, cc, m * 128:(m + 1) * 128],
                                                 ur[0:csz, cc, :],
                                                 start=(cc == 0), stop=(cc == 2))
                            nc.vector.tensor_scalar_add(
                                Q_sb[:, m, si * QT:(si + 1) * QT], pq[:, :],
                                bq_sb[:, m:m + 1])

            # ================= phase 2: attention =================
            with tc.tile_pool(name="qsb", bufs=1) as qsb, \
                 tc.tile_pool(name="qps", bufs=1, space="PSUM") as qps:
                Et = qsb.tile([128, 32, QT], F32R, name="Et")
                for qt in range(NQ // QT):
                    qsl = slice(qt * QT, (qt + 1) * QT)
                    # S^T = K^T Q (per key chunk), exp on ACT -> Et
                    for nk in range(32):
                        ps_ = qps.tile([128, QT], F32, name="ps_", tag="ps",
                                       bufs=3)
                        for cc in range(2):
                            nc.tensor.matmul(ps_[:, :],
                                             K_sb[:, cc, nk * 128:(nk + 1) * 128],
                                             Q_sb[:, cc, qsl],
                                             start=(cc == 0), stop=(cc == 1))
                        nc.scalar.activation(Et[:, nk, :], ps_[:, :], AF.Exp,
                                             scale=float(SCALE))
                    # O_aug = V~^T.T @ E^T   (last row of chunk 2 = denominators)
                    po = []
                    for cv in range(3):
                        csz = min(128, CU + 1 - cv * 128)
                        p = qps.tile([128, QT], F32, name=f"po{cv}", tag=f"po{cv}")
                        po.append(p)
                        for nk in range(32):
                            nc.tensor.matmul(p[0:csz, :],
                                             Vt_sb[:, nk, cv * 128:cv * 128 + csz],
                                             Et[:, nk, :],
                                             start=(nk == 0), stop=(nk == 31))
                    # reciprocal of denominators, broadcast to all partitions
                    nc.vector.tensor_copy(dsb[64:65, :], po[2][64:65, :])
                    pb = qps.tile([128, QT], F32, name="pb", tag="pb")
                    nc.tensor.matmul(pb[:, :], ones_mat[:, :], dsb[:, :],
                                     start=True, stop=True)
                    rb = qsb.tile([128, QT], F32, name="rb", bufs=2)
                    nc.vector.reciprocal(rb[:, :], pb[:, :])
                    # normalize
                    on = []
                    for cv in range(3):
                        csz = min(128, CU - cv * 128)
                        o_ = qsb.tile([128, QT], F32R, name=f"on{cv}",
                                      tag=f"on{cv}")
                        on.append(o_)
                        nc.vector.tensor_mul(o_[0:csz, :], po[cv][0:csz, :],
                                             rb[0:csz, :])
                    # out = Wo @ O + bo + U
                    for m in range(3):
                        msz = min(128, CU - m * 128)
                        pout = qps.tile([128, QT], F32, name="pout", tag="pout")
                        for cv in range(3):
                            csz = min(128, CU - cv * 128)
                            nc.tensor.matmul(pout[0:msz, :],
                                             WoT[0:csz, cv, m * 128:m * 128 + msz],
                                             on[cv][0:csz, :],
                                             start=(cv == 0), stop=(cv == 2))
                        f1 = qsb.tile([128, QT], F32, name="f1", tag="f1", bufs=2)
                        nc.vector.tensor_scalar_add(f1[0:msz, :], pout[0:msz, :],
                                                    bo_sb[0:msz, m:m + 1])
                        f2 = qsb.tile([128, QT], F32, name="f2", tag="f2", bufs=2)
                        nc.vector.tensor_add(f2[0:msz, :], f1[0:msz, :],
                                             U_sb[0:msz, m, qsl])
                        nc.sync.dma_start(out[m * 128:m * 128 + msz, qsl],
                                          f2[0:msz, :])

    nc.compile()
    return nc


_nc_cache = None


def _get_program():
    global _nc_cache
    if _nc_cache is None:
        _nc_cache = build_program()
    return _nc_cache


def make_in_maps(inputs):
    U = np.ascontiguousarray(np.asarray(inputs["unet_feat"], dtype=np.float32))
    J = np.ascontiguousarray(np.asarray(inputs["janus_feat"], dtype=np.float32))
    w = {k: np.ascontiguousarray(np.asarray(inputs[k], dtype=np.float32))
         for k in ("Wq", "bq", "Wk", "bk", "Wv", "bv", "Wo", "bo")}
    in_maps = []
    for core in range(NCORES):
        b, h = core // 2, core % 2
        in_maps.append({
            "u": np.ascontiguousarray(U[b].reshape(CU, N)[:, h * NQ:(h + 1) * NQ]),
            "jf": J[b].reshape(CJ, N),
            "wqT": np.ascontiguousarray(w["Wq"].T), "bq": w["bq"],
            "wkT": np.ascontiguousarray(w["Wk"].T), "bk": w["bk"],
            "wvT": np.ascontiguousarray(w["Wv"].T), "bv": w["bv"],
            "woT": np.ascontiguousarray(w["Wo"].T), "bo": w["bo"],
        })
    return in_maps


def assemble_output(results):
    out = np.empty((B, CU, N), dtype=np.float32)
    for core in range(NCORES):
        b, h = core // 2, core % 2
        out[b][:, h * NQ:(h + 1) * NQ] = results[core]["out"]
    return out.reshape(B, CU, 64, 64)


def run(inputs, trace=False, **kwargs):
    from concourse.bass_utils import run_bass_kernel_spmd
    nc = _get_program()
    res = run_bass_kernel_spmd(nc, make_in_maps(inputs),
                               core_ids=list(range(NCORES)), trace=trace,
                               **kwargs)
    return assemble_output(res.results), res


def kernel(**inputs) -> np.ndarray:
    out, _ = run(inputs, trace=False)
    return out



# revision 3
# speedup vs baseline: 2.0361x; 2.0361x over previous
"""Cross-modal attention kernel for Trainium2 (Bass/Tile), 8-core SPMD.

Reference computation (per batch b):
  q = Wq @ U + bq            U = unet_feat[b]  reshaped [320, 4096]
  k = Wk @ J + bk            J = janus_feat[b] reshaped [1024, 4096]
  v = Wv @ J + bv
  P = softmax(q^T k / 16, axis=keys)
  out = U + Wo @ (v @ P^T) + bo

Sharding: 8 cores = 4 batches x 2 query-halves. Each core computes K/V for
its batch's full key set and attention for its half of the queries.

Algebraic folds (host-side, exact):
  - bk cancels in softmax (constant per query) -> dropped.
  - Wvo = Wo @ Wv, bvo = Wo @ bv + bo:  out = (U + bvo) + Wvo J Phat^T.
  - Operands pre-scaled by powers of two into fp8 e4m3 range; compensated
    exactly in the exp scale (2^-15) and the denominator ones-column (64).

Device compute is fp8 e4m3 with MatmulPerfMode.DoubleRow (contraction 256
per matmul at 0.5 cycles/row). HW restriction: the stationary (lhsT)
operand's pair-dim must be memory-contiguous [K, 2, 128] -> all lhsT
layouts are host-arranged that way (J is uploaded in two layouts: rhs form
for the K projection, lhsT form for the V projection).

Schedule: query-tile 0's attention is fused into the J-streaming loop
(projections + attention share the 8 PSUM banks), query-tiles 1..3 follow.
exp runs on ACT (the throughput ceiling); PSUM->SBUF casts on DVE; the
residual add on Pool (GPSIMD cannot touch PSUM).
"""
import sys

if "/opt/trn_rl_repo" not in sys.path:
    sys.path.insert(0, "/opt/trn_rl_repo")

import numpy as np
import ml_dtypes

import concourse.bass as bass
import concourse.bacc as bacc
import concourse.mybir as mybir
import concourse.tile as tile

F32 = mybir.dt.float32
F32R = mybir.dt.float32r
F8 = mybir.dt.float8e4
AF = mybir.ActivationFunctionType
DR = mybir.MatmulPerfMode.DoubleRow

B = 4
C = 256        # ATTN_DIM
CU = 320
CJ = 1024
N = 4096       # H*W
NQ = N // 2    # queries per core
QT = 512       # query tile
NKP = 16       # key pair-chunks (256 keys each)
NCORES = 8

SJ, SWK, SWQ, SU, SWVO = 4.0, 8.0, 16.0, 4.0, 16.0
EXP_SCALE = (C ** -0.5) / (SWK * SJ * SWQ * SU)   # 2^-15
ONES_VAL = SWVO * SJ                              # 64.0
F8NP = ml_dtypes.float8_e4m3


def build_program():
    nc = bacc.Bacc("TRN2", target_bir_lowering=False, debug=False)

    jf8r = nc.dram_tensor("jf8r", (128, NKP, 4, 2, 256), F8,
                          kind="ExternalInput")
    jf8w = nc.dram_tensor("jf8w", (128, NKP, 8, 2, 128), F8,
                          kind="ExternalInput")
    u8 = nc.dram_tensor("u8", (128, 2, 2, NQ), F8, kind="ExternalInput")
    u32 = nc.dram_tensor("u32", (128, 3, NQ), F32, kind="ExternalInput")
    wk8 = nc.dram_tensor("wk8", (128, 4, 2, 2, 128), F8, kind="ExternalInput")
    wq8 = nc.dram_tensor("wq8", (128, 2, 2, 2, 128), F8, kind="ExternalInput")
    wvo8 = nc.dram_tensor("wvo8", (128, 4, 2, CU), F8, kind="ExternalInput")
    bq32 = nc.dram_tensor("bq32", (C,), F32, kind="ExternalInput")
    out = nc.dram_tensor("out", (CU, NQ), F32, kind="ExternalOutput")

    with tile.TileContext(nc) as tc:
        with tc.tile_pool(name="perm", bufs=1) as perm:
            # K_sb[p, ks, i, c]  = K_calc[i*128+p, ks*128+c]
            K_sb = perm.tile([128, 32, 2, 128], F8, name="K_sb")
            # Vt[p, kp, cv, i, c] = V'^T_calc[kp*256+i*128+p, cv*128+c]
            #   (cv=2: c=64 ones-column=64.0, c>64 zero pad)
            Vt_sb = perm.tile([128, NKP, 3, 2, 128], F8, name="Vt_sb")
            Q_sb = perm.tile([128, 2, NQ], F8, name="Q_sb")
            Et = perm.tile([128, 2, 2, QT], F8, name="Et")
            U32 = perm.tile([128, 3, NQ], F32, name="U32")
            ones_r = perm.tile([128, 128], F32R, name="ones_r")
            dsb = perm.tile([128, QT], F32R, name="dsb")
            bq_sb = perm.tile([128, 2], F32, name="bq_sb")

            WkT = perm.tile([128, 4, 2, 2, 128], F8, name="WkT")
            WvoT = perm.tile([128, 4, 2, CU], F8, name="WvoT")
            WqT = perm.tile([128, 2, 2, 2, 128], F8, name="WqT")
            U8 = perm.tile([128, 2, 2, NQ], F8, name="U8")

            # constants
            ones32 = perm.tile([128, 128], F32, name="ones32")
            nc.vector.memset(ones32[:, :], 1.0)
            nc.vector.tensor_copy(ones_r[:, :], ones32[:, :])
            zeros32 = perm.tile([128, QT], F32, name="zeros32")
            nc.vector.memset(zeros32[:, :], 0.0)
            nc.vector.tensor_copy(dsb[:, :], zeros32[:, :])
            nc.vector.memset(Vt_sb[:, :, 2, :, 64], ONES_VAL)
            nc.vector.memset(Vt_sb[:, :, 2, :, 65:128], 0.0)

            # loads, priority order
            nc.sync.dma_start(WqT[:, :, :, :, :], wq8[:, :, :, :, :])
            nc.sync.dma_start(U8[:, :, :, :], u8[:, :, :, :])
            nc.sync.dma_start(WkT[:, :, :, :, :], wk8[:, :, :, :, :])
            nc.sync.dma_start(WvoT[:, :, :, :], wvo8[:, :, :, :])
            for m in range(2):
                nc.sync.dma_start(bq_sb[:, m:m + 1],
                                  bq32[m * 128:(m + 1) * 128].unsqueeze(1))

            with tc.tile_pool(name="sbw", bufs=1) as sbw, \
                 tc.tile_pool(name="psA", bufs=1, space="PSUM") as psA, \
                 tc.tile_pool(name="jp", bufs=2) as jp:
                # ---- Q projection ----
                for si in range(NQ // QT):
                    qsl = slice(si * QT, (si + 1) * QT)
                    for m in range(2):
                        pq = psA.tile([128, QT], F32, name="pq", tag="ps",
                                      bufs=2)
                        for pr in range(2):
                            nc.tensor.matmul(
                                pq[:, :], WqT[:, pr, m, :, :],
                                U8[:, pr, :, qsl],
                                start=(pr == 0), stop=(pr == 1), perf_mode=DR)
                        nc.vector.tensor_scalar_add(
                            Q_sb[:, m, qsl], pq[:, :], bq_sb[:, m:m + 1])

                # ---- stream J: K/V projections fused with qt0 attention ----
                poA = [psA.tile([128, QT], F32, name=f"poA{cv}", tag=f"po{cv}")
                       for cv in range(3)]
                for t in range(NKP):
                    jtr = jp.tile([128, 4, 2, 256], F8, name="jtr")
                    nc.sync.dma_start(jtr[:, :, :, :], jf8r[:, t, :, :, :])
                    jtw = jp.tile([128, 8, 2, 128], F8, name="jtw")
                    nc.sync.dma_start(jtw[:, :, :, :], jf8w[:, t, :, :, :])
                    if t >= 13:
                        c = t - 13
                        nc.sync.dma_start(U32[:, c, :], u32[:, c, :])

                    # K projection: both m-groups into one bank
                    pk = psA.tile([128, 2, 2, 128], F32, name=f"pk{t}",
                                  tag="pk")
                    for m in range(2):
                        for pr in range(4):
                            nc.tensor.matmul(
                                pk[:, m, :, :], WkT[:, pr, m, :, :],
                                jtr[:, pr, :, :],
                                start=(pr == 0), stop=(pr == 3), perf_mode=DR)
                    # one cast for both m: view pk as (ks, m, c)
                    pk_ap = pk[:, 0, 0, :]
                    src = bass.AP(tensor=pk_ap.tensor, offset=pk_ap.offset,
                                  ap=[pk_ap.ap[0], [128, 2], [256, 2],
                                      [1, 128]])
                    nc.vector.tensor_copy(K_sb[:, 2 * t:2 * t + 2, :, :], src)

                    # V projection: both s-groups into one 2-bank tile
                    pv = psA.tile([128, 2, QT], F32, name=f"pv{t}", tag="pv")
                    for s in range(2):
                        for pr in range(4):
                            nc.tensor.matmul(
                                pv[:, s, 0:CU], jtw[:, 2 * pr + s, :, :],
                                WvoT[:, pr, :, :],
                                start=(pr == 0), stop=(pr == 3), perf_mode=DR)
                    for cv in range(3):
                        csz = min(128, CU - cv * 128)
                        nc.vector.tensor_copy(
                            Vt_sb[:, t, cv, :, 0:csz],
                            pv[:, :, cv * 128:cv * 128 + csz])

                    # qt0 scores + exp for this key chunk
                    for s in range(2):
                        ks = 2 * t + s
                        ps_ = psA.tile([128, QT], F32, name=f"ps{ks}",
                                       tag="ps", bufs=2)
                        nc.tensor.matmul(ps_[:, :], K_sb[:, ks, :, :],
                                         Q_sb[:, :, 0:QT],
                                         start=True, stop=True, perf_mode=DR)
                        nc.scalar.activation(Et[:, t % 2, s, :], ps_[:, :],
                                             AF.Exp, scale=float(EXP_SCALE))
                    for cv in range(3):
                        nc.tensor.matmul(
                            poA[cv][:, :], Vt_sb[:, t, cv, :, :],
                            Et[:, t % 2, :, :],
                            start=(t == 0), stop=(t == NKP - 1), perf_mode=DR)

                # ---- qt0 normalize + output ----
                nc.vector.tensor_copy(dsb[64:65, :], poA[2][64:65, :])
                pb = psA.tile([128, QT], F32, name="pb0", tag="pk")
                nc.tensor.matmul(pb[:, :], ones_r[:, :], dsb[:, :],
                                 start=True, stop=True)
                rb = sbw.tile([128, QT], F32, name="rb0", tag="rb", bufs=2)
                nc.vector.reciprocal(rb[:, :], pb[:, :])
                for cv in range(3):
                    csz = min(128, CU - cv * 128)
                    f1 = sbw.tile([128, QT], F32, name=f"f1_0_{cv}", tag="f1",
                                  bufs=2)
                    nc.vector.tensor_mul(f1[0:csz, :], poA[cv][0:csz, :],
                                         rb[0:csz, :])
                    f2 = sbw.tile([128, QT], F32, name=f"f2_0_{cv}", tag="f2",
                                  bufs=2)
                    nc.gpsimd.tensor_add(f2[0:csz, :], f1[0:csz, :],
                                         U32[0:csz, cv, 0:QT])
                    nc.sync.dma_start(out[cv * 128:cv * 128 + csz, 0:QT],
                                      f2[0:csz, :])

            # ---- phase B: query tiles 1..3 ----
            with tc.tile_pool(name="sbB", bufs=1) as sbB, \
                 tc.tile_pool(name="psB", bufs=1, space="PSUM") as psB:
                for qt in range(1, NQ // QT):
                    qsl = slice(qt * QT, (qt + 1) * QT)
                    po = [psB.tile([128, QT], F32, name=f"poB{qt}_{cv}",
                                   tag=f"po{cv}") for cv in range(3)]
                    for kp in range(NKP):
                        ps2 = psB.tile([128, 2, QT], F32,
                                       name=f"ps2_{qt}_{kp}", tag="ps2",
                                       bufs=2)
                        for j in range(2):
                            nc.tensor.matmul(
                                ps2[:, j, :], K_sb[:, 2 * kp + j, :, :],
                                Q_sb[:, :, qsl],
                                start=True, stop=True, perf_mode=DR)
                        nc.scalar.activation(Et[:, kp % 2, :, :],
                                             ps2[:, :, :], AF.Exp,
                                             scale=float(EXP_SCALE))
                        for cv in range(3):
                            nc.tensor.matmul(
                                po[cv][:, :], Vt_sb[:, kp, cv, :, :],
                                Et[:, kp % 2, :, :],
                                start=(kp == 0), stop=(kp == NKP - 1),
                                perf_mode=DR)
                    nc.vector.tensor_copy(dsb[64:65, :], po[2][64:65, :])
                    pb = psB.tile([128, QT], F32, name=f"pb{qt}", tag="pb")
                    nc.tensor.matmul(pb[:, :], ones_r[:, :], dsb[:, :],
                                     start=True, stop=True)
                    rb = sbB.tile([128, QT], F32, name=f"rb{qt}", tag="rb",
                                  bufs=2)
                    nc.vector.reciprocal(rb[:, :], pb[:, :])
                    for cv in range(3):
                        csz = min(128, CU - cv * 128)
                        f1 = sbB.tile([128, QT], F32, name=f"f1_{qt}_{cv}",
                                      tag="f1", bufs=2)
                        nc.vector.tensor_mul(f1[0:csz, :], po[cv][0:csz, :],
                                             rb[0:csz, :])
                        f2 = sbB.tile([128, QT], F32, name=f"f2_{qt}_{cv}",
                                      tag="f2", bufs=2)
                        nc.gpsimd.tensor_add(f2[0:csz, :], f1[0:csz, :],
                                             U32[0:csz, cv, qsl])
                        nc.sync.dma_start(out[cv * 128:cv * 128 + csz, qsl],
                                          f2[0:csz, :])

    nc.compile()
    return nc


_nc_cache = None


def _get_program():
    global _nc_cache
    if _nc_cache is None:
        _nc_cache = build_program()
    return _nc_cache


def make_in_maps(inputs):
    U = np.asarray(inputs["unet_feat"], dtype=np.float32)
    J = np.asarray(inputs["janus_feat"], dtype=np.float32)
    Wq = np.asarray(inputs["Wq"], dtype=np.float32)
    bq = np.asarray(inputs["bq"], dtype=np.float32)
    Wk = np.asarray(inputs["Wk"], dtype=np.float32)
    Wv = np.asarray(inputs["Wv"], dtype=np.float32)
    bv = np.asarray(inputs["bv"], dtype=np.float32)
    Wo = np.asarray(inputs["Wo"], dtype=np.float32)
    bo = np.asarray(inputs["bo"], dtype=np.float32)

    Wvo = (Wo.astype(np.float64) @ Wv.astype(np.float64)).astype(np.float32)
    bvo = (Wo.astype(np.float64) @ bv.astype(np.float64)).astype(np.float32) + bo

    # lhsT pair layouts: [..., pr, m, i, 128] with channel pr*256+i*128+p
    # wk8[p, pr, m, i, o] = SWK*Wk[m*128+o, pr*256+i*128+p]
    wk8 = np.ascontiguousarray(
        (SWK * Wk.T).reshape(4, 2, 128, 2, 128)
        .transpose(2, 0, 3, 1, 4)).astype(F8NP)
    wqp = np.zeros((512, 256), np.float32)
    wqp[:CU] = SWQ * Wq.T
    wq8 = np.ascontiguousarray(
        wqp.reshape(2, 2, 128, 2, 128).transpose(2, 0, 3, 1, 4)).astype(F8NP)
    # rhs pair layout for Wvo^T: [p, pr, i, cv]
    wvo8 = np.ascontiguousarray(
        (SWVO * Wvo.T).reshape(4, 2, 128, CU)
        .transpose(2, 0, 1, 3)).astype(F8NP)
    bq32 = np.ascontiguousarray(SWQ * SU * bq)

    jr_b, jw_b = [], []
    for b in range(B):
        jb = SJ * J[b].reshape(CJ, N)
        # rhs layout [p, t, pr, i, 256]
        jr_b.append(np.ascontiguousarray(
            jb.reshape(4, 2, 128, NKP, 256)
              .transpose(2, 3, 0, 1, 4)).astype(F8NP))
        # lhsT layout [p, t, pr*2+s, i, 128]
        jw_b.append(np.ascontiguousarray(
            jb.reshape(4, 2, 128, NKP, 2, 128)
              .transpose(2, 3, 0, 4, 1, 5)
              .reshape(128, NKP, 8, 2, 128)).astype(F8NP))

    in_maps = []
    for core in range(NCORES):
        b, h = core // 2, core % 2
        Up = U[b].reshape(CU, N)[:, h * NQ:(h + 1) * NQ]
        u8p = np.zeros((512, NQ), np.float32)
        u8p[:CU] = SU * Up
        u8 = np.ascontiguousarray(
            u8p.reshape(2, 2, 128, NQ).transpose(2, 0, 1, 3)).astype(F8NP)
        u32p = np.zeros((384, NQ), np.float32)
        u32p[:CU] = Up + bvo[:, None]
        u32 = np.ascontiguousarray(u32p.reshape(3, 128, NQ).transpose(1, 0, 2))
        in_maps.append({
            "jf8r": jr_b[b], "jf8w": jw_b[b], "u8": u8, "u32": u32,
            "wk8": wk8, "wq8": wq8, "wvo8": wvo8, "bq32": bq32,
        })
    return in_maps


def assemble_output(results):
    out = np.empty((B, CU, N), dtype=np.float32)
    for core in range(NCORES):
        b, h = core // 2, core % 2
        out[b][:, h * NQ:(h + 1) * NQ] = results[core]["out"]
    return out.reshape(B, CU, 64, 64)


def run(inputs, trace=False, **kwargs):
    from concourse.bass_utils import run_bass_kernel_spmd
    nc = _get_program()
    res = run_bass_kernel_spmd(nc, make_in_maps(inputs),
                               core_ids=list(range(NCORES)), trace=trace,
                               **kwargs)
    return assemble_output(res.results), res


def kernel(**inputs) -> np.ndarray:
    out, _ = run(inputs, trace=False)
    return out


# revision 4
# speedup vs baseline: 2.2099x; 1.0854x over previous
"""Cross-modal attention kernel for Trainium2 (Bass/Tile), 8-core SPMD.

Reference computation (per batch b):
  q = Wq @ U + bq            U = unet_feat[b]  reshaped [320, 4096]
  k = Wk @ J + bk            J = janus_feat[b] reshaped [1024, 4096]
  v = Wv @ J + bv
  P = softmax(q^T k / 16, axis=keys)
  out = U + Wo @ (v @ P^T) + bo

Sharding: 8 cores = 4 batches x 2 query-halves. Each core computes K/V for
its batch's full key set and attention for its half of the queries.

Algebraic folds (host-side, exact):
  - bk cancels in softmax (constant per query) -> dropped.
  - Wvo = Wo @ Wv, bvo = Wo @ bv + bo:  out = (U + bvo) + Wvo J Phat^T.
  - Operands pre-scaled by powers of two into fp8 e4m3 range; compensated
    exactly in the exp scale (2^-15) and the denominator ones-column (64).

Device compute is fp8 e4m3 with MatmulPerfMode.DoubleRow (contraction 256
per matmul at 0.5 cycles/row). HW restriction: the stationary (lhsT)
operand's pair-dim must be memory-contiguous [K, 2, 128] -> all lhsT
layouts are host-arranged that way (J is uploaded in two layouts: rhs form
for the K projection, lhsT form for the V projection).

Schedule: J streams through a software pipeline - projections for chunk t,
scores+exp for chunk t-1, O-accumulation for chunk t-2 - computing
query-tile 0's attention alongside the projections (they share the 8 PSUM
banks); query-tiles 1..3 follow with the same S/exp/O pipeline. exp runs
on ACT (the throughput ceiling); PSUM->SBUF casts on DVE; residual adds on
Pool (GPSIMD cannot touch PSUM).
"""
import sys

if "/opt/trn_rl_repo" not in sys.path:
    sys.path.insert(0, "/opt/trn_rl_repo")

import numpy as np
import ml_dtypes

import concourse.bass as bass
import concourse.bacc as bacc
import concourse.mybir as mybir
import concourse.tile as tile

F32 = mybir.dt.float32
F32R = mybir.dt.float32r
F8 = mybir.dt.float8e4
AF = mybir.ActivationFunctionType
DR = mybir.MatmulPerfMode.DoubleRow

B = 4
C = 256        # ATTN_DIM
CU = 320
CJ = 1024
N = 4096       # H*W
NQ = N // 2    # queries per core
QT = 512       # query tile
NKP = 16       # key pair-chunks (256 keys each)
NCORES = 8

SJ, SWK, SWQ, SU, SWVO = 4.0, 8.0, 16.0, 4.0, 16.0
EXP_SCALE = (C ** -0.5) / (SWK * SJ * SWQ * SU)   # 2^-15
ONES_VAL = SWVO * SJ                              # 64.0
F8NP = ml_dtypes.float8_e4m3


def build_program():
    nc = bacc.Bacc("TRN2", target_bir_lowering=False, debug=False)

    jf8r = nc.dram_tensor("jf8r", (128, NKP, 4, 2, 256), F8,
                          kind="ExternalInput")
    jf8w = nc.dram_tensor("jf8w", (128, NKP, 8, 2, 128), F8,
                          kind="ExternalInput")
    u8 = nc.dram_tensor("u8", (128, 2, 2, NQ), F8, kind="ExternalInput")
    u32 = nc.dram_tensor("u32", (128, 3, NQ), F32, kind="ExternalInput")
    wk8 = nc.dram_tensor("wk8", (128, 4, 2, 2, 128), F8, kind="ExternalInput")
    wq8 = nc.dram_tensor("wq8", (128, 2, 2, 2, 128), F8, kind="ExternalInput")
    wvo8 = nc.dram_tensor("wvo8", (128, 4, 2, CU), F8, kind="ExternalInput")
    bq32 = nc.dram_tensor("bq32", (C,), F32, kind="ExternalInput")
    out = nc.dram_tensor("out", (CU, NQ), F32, kind="ExternalOutput")

    with tile.TileContext(nc) as tc:
        with tc.tile_pool(name="perm", bufs=1) as perm:
            # K_sb[p, ks, i, c]  = K_calc[i*128+p, ks*128+c]
            K_sb = perm.tile([128, 32, 2, 128], F8, name="K_sb")
            # Vt[p, kp, cv, i, c] = V'^T_calc[kp*256+i*128+p, cv*128+c]
            #   (cv=2: c=64 ones-column=64.0, c>64 zero pad)
            Vt_sb = perm.tile([128, NKP, 3, 2, 128], F8, name="Vt_sb")
            Q_sb = perm.tile([128, 2, NQ], F8, name="Q_sb")
            Et = perm.tile([128, 4, 2, QT], F8, name="Et")
            U32 = perm.tile([128, 3, NQ], F32, name="U32")
            ones_r = perm.tile([128, 128], F32R, name="ones_r")
            dsb = perm.tile([128, QT], F32R, name="dsb")
            bq_sb = perm.tile([128, 2], F32, name="bq_sb")

            WkT = perm.tile([128, 4, 2, 2, 128], F8, name="WkT")
            WvoT = perm.tile([128, 4, 2, CU], F8, name="WvoT")
            WqT = perm.tile([128, 2, 2, 2, 128], F8, name="WqT")
            U8 = perm.tile([128, 2, 2, NQ], F8, name="U8")

            # constants
            ones32 = perm.tile([128, 128], F32, name="ones32")
            nc.vector.memset(ones32[:, :], 1.0)
            nc.vector.tensor_copy(ones_r[:, :], ones32[:, :])
            zeros32 = perm.tile([128, QT], F32, name="zeros32")
            nc.vector.memset(zeros32[:, :], 0.0)
            nc.vector.tensor_copy(dsb[:, :], zeros32[:, :])
            nc.vector.memset(Vt_sb[:, :, 2, :, 64], ONES_VAL)
            nc.vector.memset(Vt_sb[:, :, 2, :, 65:128], 0.0)

            # loads, priority order (u8 split so Q tile 0 starts early)
            nc.sync.dma_start(WqT[:, :, :, :, :], wq8[:, :, :, :, :])
            nc.sync.dma_start(U8[:, :, :, 0:QT], u8[:, :, :, 0:QT])
            nc.sync.dma_start(WkT[:, :, :, :, :], wk8[:, :, :, :, :])

            with tc.tile_pool(name="sbw", bufs=1) as sbw, \
                 tc.tile_pool(name="psA", bufs=1, space="PSUM") as psA, \
                 tc.tile_pool(name="jp", bufs=2) as jp:

                def qproj(si):
                    qsl = slice(si * QT, (si + 1) * QT)
                    for m in range(2):
                        pq = psA.tile([128, QT], F32, name=f"pq{si}_{m}",
                                      tag="ps", bufs=2)
                        for pr in range(2):
                            nc.tensor.matmul(
                                pq[:, :], WqT[:, pr, m, :, :],
                                U8[:, pr, :, qsl],
                                start=(pr == 0), stop=(pr == 1), perf_mode=DR)
                        nc.vector.tensor_scalar_add(
                            Q_sb[:, m, qsl], pq[:, :], bq_sb[:, m:m + 1])

                jts = {}

                def jload(t):
                    jtr = jp.tile([128, 4, 2, 256], F8, name=f"jtr{t}",
                                  tag="jtr")
                    nc.sync.dma_start(jtr[:, :, :, :], jf8r[:, t, :, :, :])
                    jtw = jp.tile([128, 8, 2, 128], F8, name=f"jtw{t}",
                                  tag="jtw")
                    nc.sync.dma_start(jtw[:, :, :, :], jf8w[:, t, :, :, :])
                    jts[t] = (jtr, jtw)

                # early loads interleaved with first Q projection
                for m in range(2):
                    nc.sync.dma_start(bq_sb[:, m:m + 1],
                                      bq32[m * 128:(m + 1) * 128].unsqueeze(1))
                qproj(0)
                jload(0)
                nc.sync.dma_start(WvoT[:, :, :, :], wvo8[:, :, :, :])
                jload(1)
                nc.sync.dma_start(U8[:, :, :, QT:NQ], u8[:, :, :, QT:NQ])

                # ---- stream J through the 3-stage pipeline ----
                poA = [psA.tile([128, QT], F32, name=f"poA{cv}", tag=f"po{cv}")
                       for cv in range(3)]
                for it in range(NKP + 2):
                    # stage 1: scores + exp for chunk it-1 (query tile 0)
                    tp = it - 1
                    if 0 <= tp < NKP:
                        for s in range(2):
                            ks = 2 * tp + s
                            ps_ = psA.tile([128, QT], F32, name=f"ps{ks}",
                                           tag="ps", bufs=2)
                            nc.tensor.matmul(ps_[:, :], K_sb[:, ks, :, :],
                                             Q_sb[:, :, 0:QT],
                                             start=True, stop=True,
                                             perf_mode=DR)
                            nc.scalar.activation(Et[:, tp % 4, s, :],
                                                 ps_[:, :], AF.Exp,
                                                 scale=float(EXP_SCALE))
                    # stage 0: load + projections for chunk it
                    t = it
                    if t < NKP:
                        if t + 2 < NKP:
                            jload(t + 2)
                        if t == 2:
                            nc.sync.dma_start(U32[:, 0, :], u32[:, 0, :])
                        if t in (4, 6):
                            c = t // 2 - 1
                            nc.sync.dma_start(U32[:, c, :], u32[:, c, :])
                        jtr, jtw = jts.pop(t)
                        pk = psA.tile([128, 2, 2, 128], F32, name=f"pk{t}",
                                      tag="pk")
                        for m in range(2):
                            for pr in range(4):
                                nc.tensor.matmul(
                                    pk[:, m, :, :], WkT[:, pr, m, :, :],
                                    jtr[:, pr, :, :],
                                    start=(pr == 0), stop=(pr == 3),
                                    perf_mode=DR)
                        # one cast for both m: view pk as (ks, m, c)
                        pk_ap = pk[:, 0, 0, :]
                        src = bass.AP(tensor=pk_ap.tensor, offset=pk_ap.offset,
                                      ap=[pk_ap.ap[0], [128, 2], [256, 2],
                                          [1, 128]])
                        nc.vector.tensor_copy(K_sb[:, 2 * t:2 * t + 2, :, :],
                                              src)
                        pv = psA.tile([128, 2, QT], F32, name=f"pv{t}",
                                      tag="pv")
                        for s in range(2):
                            for pr in range(4):
                                nc.tensor.matmul(
                                    pv[:, s, 0:CU], jtw[:, 2 * pr + s, :, :],
                                    WvoT[:, pr, :, :],
                                    start=(pr == 0), stop=(pr == 3),
                                    perf_mode=DR)
                        # cv0+cv1 in one copy: view pv as (cv, i, c)
                        pv_ap = pv[:, 0, 0:128]
                        srcv = bass.AP(tensor=pv_ap.tensor, offset=pv_ap.offset,
                                       ap=[pv_ap.ap[0], [128, 2], [QT, 2],
                                           [1, 128]])
                        nc.vector.tensor_copy(Vt_sb[:, t, 0:2, :, :], srcv)
                        nc.vector.tensor_copy(Vt_sb[:, t, 2, :, 0:64],
                                              pv[:, :, 256:320])
                        if t in (1, 2, 3):
                            qproj(t)
                    # stage 2: O accumulation for chunk it-2
                    to = it - 2
                    if to >= 0:
                        for cv in range(3):
                            nc.tensor.matmul(
                                poA[cv][:, :], Vt_sb[:, to, cv, :, :],
                                Et[:, to % 4, :, :],
                                start=(to == 0), stop=(to == NKP - 1),
                                perf_mode=DR)

                # ---- qt0 normalize + output ----
                nc.vector.tensor_copy(dsb[64:65, :], poA[2][64:65, :])
                pb = psA.tile([128, QT], F32, name="pb0", tag="pk")
                nc.tensor.matmul(pb[:, :], ones_r[:, :], dsb[:, :],
                                 start=True, stop=True)
                rb = sbw.tile([128, QT], F32, name="rb0", tag="rb", bufs=2)
                nc.vector.reciprocal(rb[:, :], pb[:, :])
                for cv in range(3):
                    csz = min(128, CU - cv * 128)
                    f1 = sbw.tile([128, QT], F32, name=f"f1_0_{cv}", tag="f1",
                                  bufs=2)
                    nc.vector.tensor_mul(f1[0:csz, :], poA[cv][0:csz, :],
                                         rb[0:csz, :])
                    f2 = sbw.tile([128, QT], F32, name=f"f2_0_{cv}", tag="f2",
                                  bufs=2)
                    nc.gpsimd.tensor_add(f2[0:csz, :], f1[0:csz, :],
                                         U32[0:csz, cv, 0:QT])
                    nc.sync.dma_start(out[cv * 128:cv * 128 + csz, 0:QT],
                                      f2[0:csz, :])

            # ---- phase B: query tiles 1..3, same S/exp | O pipeline ----
            with tc.tile_pool(name="sbB", bufs=1) as sbB, \
                 tc.tile_pool(name="psB", bufs=1, space="PSUM") as psB:
                for qt in range(1, NQ // QT):
                    qsl = slice(qt * QT, (qt + 1) * QT)
                    last = (qt == NQ // QT - 1)
                    po = [psB.tile([128, QT], F32, name=f"poB{qt}_{cv}",
                                   tag=f"po{cv}") for cv in range(3)]
                    for it in range(NKP + 2):
                        kp = it
                        if kp < NKP:
                            ps2 = psB.tile([128, 2, QT], F32,
                                           name=f"ps2_{qt}_{kp}", tag="ps2",
                                           bufs=2)
                            for j in range(2):
                                nc.tensor.matmul(
                                    ps2[:, j, :], K_sb[:, 2 * kp + j, :, :],
                                    Q_sb[:, :, qsl],
                                    start=True, stop=True, perf_mode=DR)
                            nc.scalar.activation(Et[:, kp % 4, :, :],
                                                 ps2[:, :, :], AF.Exp,
                                                 scale=float(EXP_SCALE))
                        ko = it - 2
                        if ko >= 0:
                            for cv in range(3):
                                nc.tensor.matmul(
                                    po[cv][:, :], Vt_sb[:, ko, cv, :, :],
                                    Et[:, ko % 4, :, :],
                                    start=(ko == 0), stop=(ko == NKP - 1),
                                    perf_mode=DR)
                    nc.vector.tensor_copy(dsb[64:65, :], po[2][64:65, :])
                    pb = psB.tile([128, QT], F32, name=f"pb{qt}", tag="pb")
                    nc.tensor.matmul(pb[:, :], ones_r[:, :], dsb[:, :],
                                     start=True, stop=True)
                    rb = sbB.tile([128, QT], F32, name=f"rb{qt}", tag="rb",
                                  bufs=2)
                    nc.vector.reciprocal(rb[:, :], pb[:, :])
                    for cv in range(3):
                        csz = min(128, CU - cv * 128)
                        f1 = sbB.tile([128, QT], F32, name=f"f1_{qt}_{cv}",
                                      tag="f1", bufs=2)
                        nc.vector.tensor_mul(f1[0:csz, :], po[cv][0:csz, :],
                                             rb[0:csz, :])
                        f2 = sbB.tile([128, QT], F32, name=f"f2_{qt}_{cv}",
                                      tag="f2", bufs=2)
                        if last:
                            nc.vector.tensor_add(f2[0:csz, :], f1[0:csz, :],
                                                 U32[0:csz, cv, qsl])
                        else:
                            nc.gpsimd.tensor_add(f2[0:csz, :], f1[0:csz, :],
                                                 U32[0:csz, cv, qsl])
                        nc.sync.dma_start(out[cv * 128:cv * 128 + csz, qsl],
                                          f2[0:csz, :])

    nc.compile()
    return nc


_nc_cache = None


def _get_program():
    global _nc_cache
    if _nc_cache is None:
        _nc_cache = build_program()
    return _nc_cache


def make_in_maps(inputs):
    U = np.asarray(inputs["unet_feat"], dtype=np.float32)
    J = np.asarray(inputs["janus_feat"], dtype=np.float32)
    Wq = np.asarray(inputs["Wq"], dtype=np.float32)
    bq = np.asarray(inputs["bq"], dtype=np.float32)
    Wk = np.asarray(inputs["Wk"], dtype=np.float32)
    Wv = np.asarray(inputs["Wv"], dtype=np.float32)
    bv = np.asarray(inputs["bv"], dtype=np.float32)
    Wo = np.asarray(inputs["Wo"], dtype=np.float32)
    bo = np.asarray(inputs["bo"], dtype=np.float32)

    Wvo = (Wo.astype(np.float64) @ Wv.astype(np.float64)).astype(np.float32)
    bvo = (Wo.astype(np.float64) @ bv.astype(np.float64)).astype(np.float32) + bo

    # lhsT pair layouts: [..., pr, m, i, 128] with channel pr*256+i*128+p
    wk8 = np.ascontiguousarray(
        (SWK * Wk.T).reshape(4, 2, 128, 2, 128)
        .transpose(2, 0, 3, 1, 4)).astype(F8NP)
    wqp = np.zeros((512, 256), np.float32)
    wqp[:CU] = SWQ * Wq.T
    wq8 = np.ascontiguousarray(
        wqp.reshape(2, 2, 128, 2, 128).transpose(2, 0, 3, 1, 4)).astype(F8NP)
    # rhs pair layout for Wvo^T: [p, pr, i, cv]
    wvo8 = np.ascontiguousarray(
        (SWVO * Wvo.T).reshape(4, 2, 128, CU)
        .transpose(2, 0, 1, 3)).astype(F8NP)
    bq32 = np.ascontiguousarray(SWQ * SU * bq)

    jr_b, jw_b = [], []
    for b in range(B):
        jb = SJ * J[b].reshape(CJ, N)
        jr_b.append(np.ascontiguousarray(
            jb.reshape(4, 2, 128, NKP, 256)
              .transpose(2, 3, 0, 1, 4)).astype(F8NP))
        jw_b.append(np.ascontiguousarray(
            jb.reshape(4, 2, 128, NKP, 2, 128)
              .transpose(2, 3, 0, 4, 1, 5)
              .reshape(128, NKP, 8, 2, 128)).astype(F8NP))

    in_maps = []
    for core in range(NCORES):
        b, h = core // 2, core % 2
        Up = U[b].reshape(CU, N)[:, h * NQ:(h + 1) * NQ]
        u8p = np.zeros((512, NQ), np.float32)
        u8p[:CU] = SU * Up
        u8c = np.ascontiguousarray(
            u8p.reshape(2, 2, 128, NQ).transpose(2, 0, 1, 3)).astype(F8NP)
        u32p = np.zeros((384, NQ), np.float32)
        u32p[:CU] = Up + bvo[:, None]
        u32c = np.ascontiguousarray(
            u32p.reshape(3, 128, NQ).transpose(1, 0, 2))
        in_maps.append({
            "jf8r": jr_b[b], "jf8w": jw_b[b], "u8": u8c, "u32": u32c,
            "wk8": wk8, "wq8": wq8, "wvo8": wvo8, "bq32": bq32,
        })
    return in_maps


def assemble_output(results):
    out = np.empty((B, CU, N), dtype=np.float32)
    for core in range(NCORES):
        b, h = core // 2, core % 2
        out[b][:, h * NQ:(h + 1) * NQ] = results[core]["out"]
    return out.reshape(B, CU, 64, 64)


def run(inputs, trace=False, **kwargs):
    from concourse.bass_utils import run_bass_kernel_spmd
    nc = _get_program()
    res = run_bass_kernel_spmd(nc, make_in_maps(inputs),
                               core_ids=list(range(NCORES)), trace=trace,
                               **kwargs)
    return assemble_output(res.results), res


def kernel(**inputs) -> np.ndarray:
    out, _ = run(inputs, trace=False)
    return out


# revision 9
# speedup vs baseline: 2.2199x; 1.0046x over previous
"""Cross-modal attention kernel for Trainium2 (Bass/Tile), 8-core SPMD.

Reference computation (per batch b):
  q = Wq @ U + bq            U = unet_feat[b]  reshaped [320, 4096]
  k = Wk @ J + bk            J = janus_feat[b] reshaped [1024, 4096]
  v = Wv @ J + bv
  P = softmax(q^T k / 16, axis=keys)
  out = U + Wo @ (v @ P^T) + bo

Sharding: 8 cores = 4 batches x 2 query-halves. Each core computes K/V for
its batch's full key set and attention for its half of the queries.

Algebraic folds (host-side, exact):
  - bk cancels in softmax (constant per query) -> dropped.
  - Wvo = Wo @ Wv, bvo = Wo @ bv + bo:  out = (U + bvo) + Wvo J Phat^T.
  - Operands pre-scaled by powers of two into fp8 e4m3 range; compensated
    exactly in the exp scale (2^-15) and the denominator ones-column (64).

Device compute is fp8 e4m3 with MatmulPerfMode.DoubleRow (contraction 256
per matmul at 0.5 cycles/row). HW restriction: the stationary (lhsT)
operand's pair-dim must be memory-contiguous [K, 2, 128] -> all lhsT
layouts are host-arranged that way (J is uploaded in two layouts: rhs form
for the K projection, lhsT form for the V projection).

Schedule: J streams through a software pipeline - projections for chunk t,
scores+exp for chunk t-1, O-accumulation for chunk t-2 - computing
query-tile 0's attention alongside the projections (they share the 8 PSUM
banks); query-tiles 1..3 follow with the same S/exp/O pipeline. exp runs
on ACT (the throughput ceiling); PSUM->SBUF casts on DVE; residual adds on
Pool (GPSIMD cannot touch PSUM).
"""
import sys

if "/opt/trn_rl_repo" not in sys.path:
    sys.path.insert(0, "/opt/trn_rl_repo")

import numpy as np
import ml_dtypes

import concourse.bass as bass
import concourse.bacc as bacc
import concourse.mybir as mybir
import concourse.tile as tile

F32 = mybir.dt.float32
F32R = mybir.dt.float32r
F8 = mybir.dt.float8e4
AF = mybir.ActivationFunctionType
DR = mybir.MatmulPerfMode.DoubleRow

B = 4
C = 256        # ATTN_DIM
CU = 320
CJ = 1024
N = 4096       # H*W
NQ = N // 2    # queries per core
QT = 512       # query tile
NKP = 16       # key pair-chunks (256 keys each)
NCORES = 8

SJ, SWK, SWQ, SU, SWVO = 4.0, 8.0, 16.0, 4.0, 16.0
EXP_SCALE = (C ** -0.5) / (SWK * SJ * SWQ * SU)   # 2^-15
ONES_VAL = SWVO * SJ                              # 64.0
F8NP = ml_dtypes.float8_e4m3


def build_program():
    nc = bacc.Bacc("TRN2", target_bir_lowering=False, debug=False)

    jf8r = nc.dram_tensor("jf8r", (128, NKP, 4, 2, 256), F8,
                          kind="ExternalInput")
    jf8w = nc.dram_tensor("jf8w", (128, NKP, 8, 2, 128), F8,
                          kind="ExternalInput")
    u8 = nc.dram_tensor("u8", (128, 2, 2, NQ), F8, kind="ExternalInput")
    u32 = nc.dram_tensor("u32", (128, 3, NQ), F32, kind="ExternalInput")
    wk8 = nc.dram_tensor("wk8", (128, 4, 2, 2, 128), F8, kind="ExternalInput")
    wq8 = nc.dram_tensor("wq8", (128, 2, 2, 2, 128), F8, kind="ExternalInput")
    wvo8 = nc.dram_tensor("wvo8", (128, 4, 2, CU), F8, kind="ExternalInput")
    bq32 = nc.dram_tensor("bq32", (C,), F32, kind="ExternalInput")
    out = nc.dram_tensor("out", (CU, NQ), F32, kind="ExternalOutput")

    with tile.TileContext(nc) as tc:
        with tc.tile_pool(name="perm", bufs=1) as perm:
            # K_sb[p, ks, i, c]  = K_calc[i*128+p, ks*128+c]
            K_sb = perm.tile([128, 32, 2, 128], F8, name="K_sb")
            # Vt[p, kp, cv, i, c] = V'^T_calc[kp*256+i*128+p, cv*128+c]
            #   (cv=2: c=64 ones-column=64.0, c>64 zero pad)
            Vt_sb = perm.tile([128, NKP, 3, 2, 128], F8, name="Vt_sb")
            Q_sb = perm.tile([128, 2, NQ], F8, name="Q_sb")
            Et = perm.tile([128, 4, 2, QT], F8, name="Et")
            U32 = perm.tile([128, 3, NQ], F32, name="U32")
            ones_r = perm.tile([128, 128], F32R, name="ones_r")
            dsb = perm.tile([128, QT], F32R, name="dsb")
            bq_sb = perm.tile([128, 2], F32, name="bq_sb")

            WkT = perm.tile([128, 4, 2, 2, 128], F8, name="WkT")
            WvoT = perm.tile([128, 4, 2, CU], F8, name="WvoT")
            WqT = perm.tile([128, 2, 2, 2, 128], F8, name="WqT")
            U8 = perm.tile([128, 2, 2, NQ], F8, name="U8")

            # constants
            ones32 = perm.tile([128, 128], F32, name="ones32")
            nc.vector.memset(ones32[:, :], 1.0)
            nc.vector.tensor_copy(ones_r[:, :], ones32[:, :])
            zeros32 = perm.tile([128, QT], F32, name="zeros32")
            nc.vector.memset(zeros32[:, :], 0.0)
            nc.vector.tensor_copy(dsb[:, :], zeros32[:, :])
            nc.vector.memset(Vt_sb[:, :, 2, :, 64], ONES_VAL)
            nc.vector.memset(Vt_sb[:, :, 2, :, 65:128], 0.0)

            # loads, priority order (u8 split so Q tile 0 starts early)
            nc.sync.dma_start(WqT[:, :, :, :, :], wq8[:, :, :, :, :])
            nc.sync.dma_start(U8[:, :, :, 0:QT], u8[:, :, :, 0:QT])
            nc.sync.dma_start(WkT[:, :, :, :, :], wk8[:, :, :, :, :])

            with tc.tile_pool(name="sbw", bufs=1) as sbw, \
                 tc.tile_pool(name="psA", bufs=1, space="PSUM") as psA, \
                 tc.tile_pool(name="jp", bufs=2) as jp:

                def qproj(si):
                    qsl = slice(si * QT, (si + 1) * QT)
                    for m in range(2):
                        pq = psA.tile([128, QT], F32, name=f"pq{si}_{m}",
                                      tag="ps", bufs=3)
                        for pr in range(2):
                            nc.tensor.matmul(
                                pq[:, :], WqT[:, pr, m, :, :],
                                U8[:, pr, :, qsl],
                                start=(pr == 0), stop=(pr == 1), perf_mode=DR)
                        nc.vector.tensor_scalar_add(
                            Q_sb[:, m, qsl], pq[:, :], bq_sb[:, m:m + 1])

                jts = {}

                def jload(t):
                    jtr = jp.tile([128, 4, 2, 256], F8, name=f"jtr{t}",
                                  tag="jtr")
                    nc.sync.dma_start(jtr[:, :, :, :], jf8r[:, t, :, :, :])
                    jtw = jp.tile([128, 8, 2, 128], F8, name=f"jtw{t}",
                                  tag="jtw")
                    nc.sync.dma_start(jtw[:, :, :, :], jf8w[:, t, :, :, :])
                    jts[t] = (jtr, jtw)

                # early loads interleaved with first Q projection
                for m in range(2):
                    nc.sync.dma_start(bq_sb[:, m:m + 1],
                                      bq32[m * 128:(m + 1) * 128].unsqueeze(1))
                qproj(0)
                jload(0)
                nc.sync.dma_start(WvoT[:, :, :, :], wvo8[:, :, :, :])
                jload(1)
                nc.sync.dma_start(U8[:, :, :, QT:NQ], u8[:, :, :, QT:NQ])

                # ---- stream J through the 3-stage pipeline ----
                poA = [psA.tile([128, QT], F32, name=f"poA{cv}", tag=f"po{cv}")
                       for cv in range(3)]
                for it in range(NKP + 2):
                    # stage 1: scores + exp for chunk it-1 (query tile 0)
                    tp = it - 1
                    if 0 <= tp < NKP:
                        for s in range(2):
                            ks = 2 * tp + s
                            ps_ = psA.tile([128, QT], F32, name=f"ps{ks}",
                                           tag="ps", bufs=3)
                            nc.tensor.matmul(ps_[:, :], K_sb[:, ks, :, :],
                                             Q_sb[:, :, 0:QT],
                                             start=True, stop=True,
                                             perf_mode=DR)
                            nc.scalar.activation(Et[:, tp % 4, s, :],
                                                 ps_[:, :], AF.Exp,
                                                 scale=float(EXP_SCALE))
                    # stage 0: load + projections for chunk it
                    t = it
                    if t < NKP:
                        if t + 2 < NKP:
                            jload(t + 2)
                        if t in (2, 4, 6):
                            c = t // 2 - 1
                            nc.sync.dma_start(U32[:, c, :], u32[:, c, :])
                        jtr, jtw = jts.pop(t)
                        # K staging shares the "ps" tag bank rotation
                        pk = psA.tile([128, 2, 2, 128], F32, name=f"pk{t}",
                                      tag="ps", bufs=3)
                        for m in range(2):
                            for pr in range(4):
                                nc.tensor.matmul(
                                    pk[:, m, :, :], WkT[:, pr, m, :, :],
                                    jtr[:, pr, :, :],
                                    start=(pr == 0), stop=(pr == 3),
                                    perf_mode=DR)
                        # one cast for both m: view pk as (ks, m, c)
                        pk_ap = pk[:, 0, 0, :]
                        src = bass.AP(tensor=pk_ap.tensor, offset=pk_ap.offset,
                                      ap=[pk_ap.ap[0], [128, 2], [256, 2],
                                          [1, 128]])
                        nc.vector.tensor_copy(K_sb[:, 2 * t:2 * t + 2, :, :],
                                              src)
                        pv = psA.tile([128, 2, QT], F32, name=f"pv{t}",
                                      tag="pv", bufs=1)
                        for s in range(2):
                            for pr in range(4):
                                nc.tensor.matmul(
                                    pv[:, s, 0:CU], jtw[:, 2 * pr + s, :, :],
                                    WvoT[:, pr, :, :],
                                    start=(pr == 0), stop=(pr == 3),
                                    perf_mode=DR)
                        # cv0+cv1 in one copy: view pv as (cv, i, c)
                        pv_ap = pv[:, 0, 0:128]
                        srcv = bass.AP(tensor=pv_ap.tensor, offset=pv_ap.offset,
                                       ap=[pv_ap.ap[0], [128, 2], [QT, 2],
                                           [1, 128]])
                        nc.vector.tensor_copy(Vt_sb[:, t, 0:2, :, :], srcv)
                        nc.scalar.copy(Vt_sb[:, t, 2, :, 0:64],
                                       pv[:, :, 256:320])
                        if t in (1, 2, 3):
                            qproj(t)
                    # stage 2: O accumulation for chunk it-2
                    to = it - 2
                    if to >= 0:
                        for cv in range(3):
                            nc.tensor.matmul(
                                poA[cv][:, :], Vt_sb[:, to, cv, :, :],
                                Et[:, to % 4, :, :],
                                start=(to == 0), stop=(to == NKP - 1),
                                perf_mode=DR)

                # ---- qt0 normalize + output ----
                nc.vector.tensor_copy(dsb[64:65, :], poA[2][64:65, :])
                pb = psA.tile([128, QT], F32, name="pb0", tag="ps", bufs=3)
                nc.tensor.matmul(pb[:, :], ones_r[:, :], dsb[:, :],
                                 start=True, stop=True)
                rb = sbw.tile([128, QT], F32, name="rb0", tag="rb", bufs=2)
                nc.vector.reciprocal(rb[:, :], pb[:, :])
                for cv in range(3):
                    csz = min(128, CU - cv * 128)
                    f1 = sbw.tile([128, QT], F32, name=f"f1_0_{cv}", tag="f1",
                                  bufs=2)
                    nc.vector.tensor_mul(f1[0:csz, :], poA[cv][0:csz, :],
                                         rb[0:csz, :])
                    f2 = sbw.tile([128, QT], F32, name=f"f2_0_{cv}", tag="f2",
                                  bufs=2)
                    if cv == 1:
                        nc.vector.tensor_add(f2[0:csz, :], f1[0:csz, :],
                                             U32[0:csz, cv, 0:QT])
                    else:
                        nc.gpsimd.tensor_add(f2[0:csz, :], f1[0:csz, :],
                                             U32[0:csz, cv, 0:QT])
                    nc.sync.dma_start(out[cv * 128:cv * 128 + csz, 0:QT],
                                      f2[0:csz, :])

            # ---- phase B: query tiles 1..3, same S/exp | O pipeline ----
            with tc.tile_pool(name="sbB", bufs=1) as sbB, \
                 tc.tile_pool(name="psB", bufs=1, space="PSUM") as psB:
                for qt in range(1, NQ // QT):
                    qsl = slice(qt * QT, (qt + 1) * QT)
                    last = (qt == NQ // QT - 1)
                    po = None
                    for it in range(NKP + 2):
                        kp = it
                        if kp < NKP:
                            ps2 = psB.tile([128, 2, QT], F32,
                                           name=f"ps2_{qt}_{kp}", tag="ps2",
                                           bufs=2)
                            for j in range(2):
                                nc.tensor.matmul(
                                    ps2[:, j, :], K_sb[:, 2 * kp + j, :, :],
                                    Q_sb[:, :, qsl],
                                    start=True, stop=True, perf_mode=DR)
                            nc.scalar.activation(Et[:, kp % 4, :, :],
                                                 ps2[:, :, :], AF.Exp,
                                                 scale=float(EXP_SCALE))
                        ko = it - 2
                        if ko >= 0:
                            if po is None:
                                # allocated after ps2 so ps2's banks alias
                                # phase A's earliest-freed region
                                po = [psB.tile([128, QT], F32,
                                               name=f"poB{qt}_{cv}",
                                               tag=f"po{cv}")
                                      for cv in range(3)]
                            for cv in range(3):
                                nc.tensor.matmul(
                                    po[cv][:, :], Vt_sb[:, ko, cv, :, :],
                                    Et[:, ko % 4, :, :],
                                    start=(ko == 0), stop=(ko == NKP - 1),
                                    perf_mode=DR)
                    nc.vector.tensor_copy(dsb[64:65, :], po[2][64:65, :])
                    pb = psB.tile([128, QT], F32, name=f"pb{qt}", tag="pb")
                    nc.tensor.matmul(pb[:, :], ones_r[:, :], dsb[:, :],
                                     start=True, stop=True)
                    rb = sbB.tile([128, QT], F32, name=f"rb{qt}", tag="rb",
                                  bufs=2)
                    nc.vector.reciprocal(rb[:, :], pb[:, :])
                    for cv in range(3):
                        csz = min(128, CU - cv * 128)
                        f1 = sbB.tile([128, QT], F32, name=f"f1_{qt}_{cv}",
                                      tag="f1", bufs=2)
                        nc.vector.tensor_mul(f1[0:csz, :], po[cv][0:csz, :],
                                             rb[0:csz, :])
                        f2 = sbB.tile([128, QT], F32, name=f"f2_{qt}_{cv}",
                                      tag="f2", bufs=2)
                        if cv == 1 or (last and cv == 2):
                            nc.vector.tensor_add(f2[0:csz, :], f1[0:csz, :],
                                                 U32[0:csz, cv, qsl])
                        else:
                            nc.gpsimd.tensor_add(f2[0:csz, :], f1[0:csz, :],
                                                 U32[0:csz, cv, qsl])
                        nc.sync.dma_start(out[cv * 128:cv * 128 + csz, qsl],
                                          f2[0:csz, :])

    nc.compile()
    return nc


_nc_cache = None


def _get_program():
    global _nc_cache
    if _nc_cache is None:
        _nc_cache = build_program()
    return _nc_cache


def make_in_maps(inputs):
    U = np.asarray(inputs["unet_feat"], dtype=np.float32)
    J = np.asarray(inputs["janus_feat"], dtype=np.float32)
    Wq = np.asarray(inputs["Wq"], dtype=np.float32)
    bq = np.asarray(inputs["bq"], dtype=np.float32)
    Wk = np.asarray(inputs["Wk"], dtype=np.float32)
    Wv = np.asarray(inputs["Wv"], dtype=np.float32)
    bv = np.asarray(inputs["bv"], dtype=np.float32)
    Wo = np.asarray(inputs["Wo"], dtype=np.float32)
    bo = np.asarray(inputs["bo"], dtype=np.float32)

    Wvo = (Wo.astype(np.float64) @ Wv.astype(np.float64)).astype(np.float32)
    bvo = (Wo.astype(np.float64) @ bv.astype(np.float64)).astype(np.float32) + bo

    # lhsT pair layouts: [..., pr, m, i, 128] with channel pr*256+i*128+p
    wk8 = np.ascontiguousarray(
        (SWK * Wk.T).reshape(4, 2, 128, 2, 128)
        .transpose(2, 0, 3, 1, 4)).astype(F8NP)
    wqp = np.zeros((512, 256), np.float32)
    wqp[:CU] = SWQ * Wq.T
    wq8 = np.ascontiguousarray(
        wqp.reshape(2, 2, 128, 2, 128).transpose(2, 0, 3, 1, 4)).astype(F8NP)
    # rhs pair layout for Wvo^T: [p, pr, i, cv]
    wvo8 = np.ascontiguousarray(
        (SWVO * Wvo.T).reshape(4, 2, 128, CU)
        .transpose(2, 0, 1, 3)).astype(F8NP)
    bq32 = np.ascontiguousarray(SWQ * SU * bq)

    jr_b, jw_b = [], []
    for b in range(B):
        jb = SJ * J[b].reshape(CJ, N)
        jr_b.append(np.ascontiguousarray(
            jb.reshape(4, 2, 128, NKP, 256)
              .transpose(2, 3, 0, 1, 4)).astype(F8NP))
        jw_b.append(np.ascontiguousarray(
            jb.reshape(4, 2, 128, NKP, 2, 128)
              .transpose(2, 3, 0, 4, 1, 5)
              .reshape(128, NKP, 8, 2, 128)).astype(F8NP))

    in_maps = []
    for core in range(NCORES):
        b, h = core // 2, core % 2
        Up = U[b].reshape(CU, N)[:, h * NQ:(h + 1) * NQ]
        u8p = np.zeros((512, NQ), np.float32)
        u8p[:CU] = SU * Up
        u8c = np.ascontiguousarray(
            u8p.reshape(2, 2, 128, NQ).transpose(2, 0, 1, 3)).astype(F8NP)
        u32p = np.zeros((384, NQ), np.float32)
        u32p[:CU] = Up + bvo[:, None]
        u32c = np.ascontiguousarray(
            u32p.reshape(3, 128, NQ).transpose(1, 0, 2))
        in_maps.append({
            "jf8r": jr_b[b], "jf8w": jw_b[b], "u8": u8c, "u32": u32c,
            "wk8": wk8, "wq8": wq8, "wvo8": wvo8, "bq32": bq32,
        })
    return in_maps


def assemble_output(results):
    out = np.empty((B, CU, N), dtype=np.float32)
    for core in range(NCORES):
        b, h = core // 2, core % 2
        out[b][:, h * NQ:(h + 1) * NQ] = results[core]["out"]
    return out.reshape(B, CU, 64, 64)


def run(inputs, trace=False, **kwargs):
    from concourse.bass_utils import run_bass_kernel_spmd
    nc = _get_program()
    res = run_bass_kernel_spmd(nc, make_in_maps(inputs),
                               core_ids=list(range(NCORES)), trace=trace,
                               **kwargs)
    return assemble_output(res.results), res


def kernel(**inputs) -> np.ndarray:
    out, _ = run(inputs, trace=False)
    return out
